# revision 56
# baseline (speedup 1.0000x reference)
"""DynamicGraphCNN (DGCNN) forward pass on 8 Trainium2 NeuronCores.

Data-parallel over batch B=8: one point cloud per core. Per layer (edge-conv):
  scores  S'[i,j] = <x_i, x_j> - ||x_j||^2/2    (rank-equivalent to -dist^2)
  top-20 neighbors per row via DVE max/max_index/match_replace
  h[i,j] = u_i + v_{n(i,j)} with u = x(Wc-Wn)^T + b, v = x Wn^T
  BN (training stats over B,N,k) from global sums:
      Sum h   = 20*Sum u + Sum_{ij} v_n
      Sum h^2 = 20*Sum u^2 + 2*Sum_i u_i.s_i + Sum_{ij} v_n^2
  computed with bf16 PE matmuls over the gathered tiles (j-packed into psum),
  cross-term via u-weighted matmuls + diagonal-mask extraction.
  Cross-core reduction: one 8-core AllReduce per layer.
  y_i = relu(scale*(u_i + max_j v_n) + shift)   (monotone: max before affine)
Final: global max over points, then linear head.

v2: negxx fused into the score matmul via an augmented lhs row (L1/L2),
4-way SWDGE queue spread for the gathers, bf16 gather tables for L2/L3
(halving gather DMA and enabling 2x bf16 max-trees), stats matmuls grouped
by stationary operand to reuse loaded weights.
"""
import sys
sys.path.insert(0, '/opt/trn_rl_repo')

import numpy as np

B, N, K = 8, 2048, 20
NT = N // 128                      # 16 point tiles of 128
LAYERS = [(3, 64), (64, 128), (128, 256)]
NCORES = 8
GATHER_SPLITS = [(0, 640), (640, 640), (1280, 640), (1920, 640)]

_BUILT = {}


def _build(dbg=False):
    import contextlib
    import concourse.bacc as bacc
    import concourse.mybir as mybir
    import concourse.tile as tile

    f32 = mybir.dt.float32
    f32r = mybir.dt.float32r
    bf16 = mybir.dt.bfloat16
    i16 = mybir.dt.int16
    f16 = mybir.dt.float16
    u32 = mybir.dt.uint32
    AOT = mybir.AluOpType
    AF = mybir.ActivationFunctionType

    nc = bacc.Bacc("TRN2", target_bir_lowering=False, debug=False,
                   num_devices=NCORES, num_swdge_queues=4)

    # ---------------- external tensors ----------------
    xT_in = nc.dram_tensor("xT", [3, N], f32, kind="ExternalInput")
    ext = {}
    for li, (ci, co) in enumerate(LAYERS):
        # wuv = [wcm | wn] (u and v weights side by side), pre-split hi/lo
        # bf16 on the host for the 3-term bf16 matmuls; brw = [bias_hi;
        # bias_lo] rows (v half zero).
        ext[f"wuvh{li}"] = nc.dram_tensor(f"wuvh{li}", [ci, 2 * co], bf16, kind="ExternalInput")
        ext[f"wuvl{li}"] = nc.dram_tensor(f"wuvl{li}", [ci, 2 * co], bf16, kind="ExternalInput")
        ext[f"brw{li}"] = nc.dram_tensor(f"brw{li}", [2, 2 * co], bf16, kind="ExternalInput")
        for rn in ("grow", "berow"):
            ext[f"{rn}{li}"] = nc.dram_tensor(f"{rn}{li}", [1, co], f32, kind="ExternalInput")
        for h in range(-(-co // 128)):
            hc = min(128, co - 128 * h)
            ext[f"mask{li}_{h}"] = nc.dram_tensor(
                f"mask{li}_{h}", [hc, 512], f32, kind="ExternalInput")
    ident_in = nc.dram_tensor("ident", [128, 128], f32, kind="ExternalInput")
    rep16_in = nc.dram_tensor("rep16", [16, 128], mybir.dt.float16, kind="ExternalInput")
    woT_in = nc.dram_tensor("woT", [256, 256], f32, kind="ExternalInput")
    bo_in = nc.dram_tensor("boRow", [1, 256], f32, kind="ExternalInput")
    out_ext = nc.dram_tensor("out", [1, 256], f32, kind="ExternalOutput")

    with tile.TileContext(nc) as tc:
        ctx = contextlib.ExitStack()
        with ctx:
            big = ctx.enter_context(tc.tile_pool(name="big", bufs=3))      # S / ysq
            ytp = ctx.enter_context(tc.tile_pool(name="ytp", bufs=1))      # yT (2 tags)
            allp = ctx.enter_context(tc.tile_pool(name="allp", bufs=1))    # layer residents
            resid = ctx.enter_context(tc.tile_pool(name="resid", bufs=1))  # constants
            dstp = ctx.enter_context(tc.tile_pool(name="dstp", bufs=3))
            bfp = ctx.enter_context(tc.tile_pool(name="bfp", bufs=2))      # dsq / trees
            small = ctx.enter_context(tc.tile_pool(name="small", bufs=2))  # idx plumbing
            rows = ctx.enter_context(tc.tile_pool(name="rows", bufs=1))    # [1,*] rows
            vcp = ctx.enter_context(tc.tile_pool(name="vcp", bufs=2))      # staging
            dram = ctx.enter_context(tc.tile_pool(name="dram", bufs=1, space="DRAM"))
            pscore = ctx.enter_context(tc.tile_pool(name="pscore", bufs=2, space="PSUM"))
            pyp = ctx.enter_context(tc.tile_pool(name="pyp", bufs=1, space="PSUM"))
            pmix = ctx.enter_context(tc.tile_pool(name="pmix", bufs=2, space="PSUM"))
            pstat = ctx.enter_context(tc.tile_pool(name="pstat", bufs=1, space="PSUM"))

            # ---------- kernel-lifetime constants ----------
            ident = resid.tile([128, 128], f32, tag="ident")
            nc.sync.dma_start(ident[:], ident_in[:])
            rep16 = resid.tile([16, 128], f16, tag="rep16")
            nc.sync.dma_start(rep16[:], rep16_in[:])
            onesRow = resid.tile([1, 128], f32, tag="onesRow")
            nc.vector.memset(onesRow[:], 1.0)
            ones2 = resid.tile([2, 128], bf16, tag="ones2")
            nc.vector.memset(ones2[:], 1.0)
            # picks rows 0 (negxx hi) and 32 (negxx lo) out of nxB
            onesNx = resid.tile([33, 128], bf16, tag="onesNx")
            nc.vector.memset(onesNx[:], 0.0)
            nc.vector.memset(onesNx[0:1, :], 1.0)
            nc.vector.memset(onesNx[32:33, :], 1.0)
            onesColF = resid.tile([128, 1], f32, tag="onesColF")
            nc.vector.memset(onesColF[:], 1.0)
            onesCol_bf = resid.tile([128, 1], bf16, tag="onesColbf")
            nc.vector.memset(onesCol_bf[:], 1.0)
            negHalfCol = resid.tile([128, 1], f32, tag="negHalfCol")
            nc.vector.memset(negHalfCol[:], -0.5)
            woT_sb = resid.tile([128, 2, 256], f32, tag="woT")
            for h in range(2):
                nc.sync.dma_start(woT_sb[:, h, :], woT_in[128 * h:128 * (h + 1), :])
            boRow = resid.tile([1, 256], f32, tag="boRow")
            nc.sync.dma_start(boRow[:], bo_in[:])
            gmax = resid.tile([128, 2, 128], f32, tag="gmax")
            nc.vector.memset(gmax[:], -1e30)
            # per-partition column-index row 0..2047, used to embed j into the
            # low 11 mantissa bits of the scores (index recovery without
            # find_index8 passes)
            iota2048 = resid.tile([128, N], u32, tag="iota2048")
            nc.gpsimd.iota(iota2048[:], pattern=[[1, N]], base=0,
                           channel_multiplier=0)
            maskHi = resid.tile([128, 1], u32, tag="maskHi")
            nc.vector.memset(maskHi[:], 0xFFFFF800)
            maskLo = resid.tile([128, 1], u32, tag="maskLo")
            nc.vector.memset(maskLo[:], 0x7FF)

            # yT carries the CI feature rows plus one negxx row for L1/L2 so
            # the -||x_j||^2/2 bias rides the score matmul as an extra
            # contraction row. Engine accesses need 32-aligned base
            # partitions, so L1 (CI=3) pads rows 3..31 with zeros and puts
            # negxx at row 32; L2 (CI=64) puts it at row 64. L3 (CI=128) has
            # no spare partition and keeps the separate bias matmul.
            yT = ytp.tile([128, N], f32, tag="yt0")
            nc.vector.memset(yT[0:33, :], 0.0)
            nc.sync.dma_start(yT[0:3, :], xT_in[:])

            for li, (CI, CO) in enumerate(LAYERS):
                NH = -(-CO // 128)
                CH = min(128, CO)
                last_layer = (li == len(LAYERS) - 1)
                # L1/L2 gather [v | v^2] rows (sq_tab): doubles the row to
                # 512B (full DMA line rate vs the <512B half-rate penalty,
                # so same gather time) and kills the per-tile dsq Square.
                # L3 keeps plain v rows (doubling them would double an
                # already-full-rate 512B gather).
                sq_tab = not last_layer
                CO2 = 2 * CO if sq_tab else CO
                # L1 keeps exact f32 v (and 2*64*4 = 512B full-rate rows);
                # L2/L3 use bf16 tables (512B rows for L2's [v|v^2]).
                gdt = f32 if li == 0 else bf16
                G = 512 // CO2
                jgroups = []
                j0 = 0
                while j0 < K:
                    jgroups.append((j0, min(G, K - j0)))
                    j0 += G

                # ---------- weights / rows ----------
                wuvh = allp.tile([CI, 2 * CO], bf16, tag="wuvh")
                nc.sync.dma_start(wuvh[:], ext[f"wuvh{li}"][:])
                wuvl = allp.tile([CI, 2 * CO], bf16, tag="wuvl")
                nc.sync.dma_start(wuvl[:], ext[f"wuvl{li}"][:])
                brw = allp.tile([2, 2 * CO], bf16, tag="brw")
                nc.sync.dma_start(brw[:], ext[f"brw{li}"][:])
                grow = allp.tile([1, CO], f32, tag="grow")
                nc.sync.dma_start(grow[:], ext[f"grow{li}"][:])
                berow = allp.tile([1, CO], f32, tag="berow")
                nc.sync.dma_start(berow[:], ext[f"berow{li}"][:])
                masks = []
                for h in range(NH):
                    mk = allp.tile([CH, 512], f32, tag=f"mask{h}")
                    nc.sync.dma_start(mk[:], ext[f"mask{li}_{h}"][:])
                    masks.append(mk)

                # ---------- prep: bf16 hi/lo score operands + negxx ----------
                # Scores run as 3-term bf16 matmuls (hi*hi + hi*lo + lo*hi,
                # dropping lo*lo ~ 2^-18): bf16 streams at 1 cyc/row on PE
                # vs fp32's 4, and ranking error stays far below the 2^-12
                # index-embedding quantization. negxx is computed exactly in
                # fp32 then split hi/lo the same way. Layouts (32-aligned
                # base partitions for every engine access):
                #   L1: yB2 = [hi@0, lo@32, hi@64, nxHi@96, nxLo@97] (1 mm)
                #   L2: yB = [hi@0, lo@64]; yB2 = [hi@0, nxHi@64, nxLo@65]
                #   L3: yB = hi, yB2 = lo, nxB = [nxHi, nxLo]  (4 mms)
                ysq = big.tile([128, N], f32, tag="big")
                nc.scalar.activation(ysq[0:CI, :], yT[0:CI, :], AF.Square)
                nxF = allp.tile([1, N], f32, tag="nxF")
                for nj in range(4):
                    ps = pmix.tile([1, 512], f32, tag="pmix")
                    nc.tensor.matmul(ps[:], negHalfCol[0:CI, :],
                                     ysq[0:CI, nj * 512:(nj + 1) * 512],
                                     start=True, stop=True)
                    nc.scalar.activation(nxF[:, nj * 512:(nj + 1) * 512],
                                         ps[:], AF.Copy)
                yB = allp.tile([128, N], bf16, tag="yB")
                yB2 = allp.tile([128, N], bf16, tag="yB2")
                nxB = allp.tile([33, N], bf16, tag="nxB")
                if li == 0:
                    nc.vector.memset(yB[0:96, :], 0.0)
                hi_dst = yB[0:CI, :]
                nc.scalar.activation(hi_dst, yT[0:CI, :], AF.Copy)
                yLoF = big.tile([128, N], f32, tag="big", name=f"yLoF{li}")
                nc.vector.tensor_tensor(out=yLoF[0:CI, :], in0=yT[0:CI, :],
                                        in1=hi_dst, op=AOT.subtract)
                # lo always lands at base partition 0 in yB2 (matmul lhsT and
                # rhs must share base_partition), plus packed copies inside yB
                # where the single-matmul rhs layouts need them.
                nc.scalar.activation(yB2[0:CI, :], yLoF[0:CI, :], AF.Copy)
                if li == 0:
                    # packed rhs layout: lo at rows 32-34, hi again at 64-66
                    nc.scalar.activation(yB[32:35, :], yLoF[0:CI, :], AF.Copy)
                    nc.scalar.activation(yB[64:67, :], yT[0:CI, :], AF.Copy)
                elif li == 1:
                    nc.scalar.activation(yB[64:128, :], yLoF[0:CI, :], AF.Copy)
                nc.scalar.activation(nxB[0:1, :], nxF[:], AF.Copy)
                nxLoF = allp.tile([1, N], f32, tag="nxLoF")
                nc.vector.tensor_tensor(out=nxLoF[:], in0=nxF[:],
                                        in1=nxB[0:1, :], op=AOT.subtract)
                nc.scalar.activation(nxB[32:33, :], nxLoF[:], AF.Copy)

                # ---------- stat psums (locked for the layer) ----------
                psSQu = pstat.tile([33, 512], f32, tag="psSQ", name="psSQu")
                psSu = psSQu[0:1, :]
                psQu = psSQu[32:33, :]
                psX = [pstat.tile([CH, 512], f32, tag=f"psX{h}", name=f"psX{li}_{h}") for h in range(NH)]

                yPre = None
                if not last_layer:
                    yPre = big.tile([CH, N], f32, tag="big", name=f"yPre{li}")

                vtab = dram.tile([N, CO2], gdt, tag=f"vtab{li}")
                u_all = allp.tile([128, NT, CO], f32, tag="u_all")
                ubf_all = allp.tile([128, NT, CO], bf16, tag="ubf_all")

                st = {}   # per-tile tiles: S, maxv, idxu, dst

                def scores_phase(t):
                    tsl = slice(t * 128, (t + 1) * 128)
                    S = big.tile([128, N], f32, tag="big", name=f"S{li}_{t}")
                    if li == 0:
                        aug = small.tile([96, 128], bf16, tag="aug0",
                                         name=f"aug0_{t}")
                        if t < 2:
                            nc.vector.memset(aug[:], 0.0)
                        nc.scalar.activation(aug[0:3, :], yB[0:3, tsl], AF.Copy)
                        nc.scalar.activation(aug[32:35, :], yB[0:3, tsl], AF.Copy)
                        nc.scalar.activation(aug[64:67, :], yB[32:35, tsl], AF.Copy)
                    elif li == 1:
                        # [hi_i; hi_i] packed so one matmul covers hi*hi+hi*lo
                        aug = small.tile([128, 128], bf16, tag="aug1",
                                         name=f"aug1_{t}")
                        nc.scalar.activation(aug[0:64, :], yB[0:64, tsl], AF.Copy)
                        nc.scalar.activation(aug[64:128, :], yB[0:64, tsl], AF.Copy)
                    for nj in range(4):
                        c = slice(nj * 512, (nj + 1) * 512)
                        psSc = pscore.tile([128, 512], f32, tag="psc")
                        if li == 0:
                            nc.tensor.matmul(psSc[:], aug[:], yB[0:96, c],
                                             start=True, stop=False)
                        elif li == 1:
                            nc.tensor.matmul(psSc[:], aug[:], yB[:, c],
                                             start=True, stop=False)
                            nc.tensor.matmul(psSc[:], yB2[0:64, tsl], yB[0:64, c],
                                             start=False, stop=False)
                        else:
                            nc.tensor.matmul(psSc[:], yB[:, tsl], yB[:, c],
                                             start=True, stop=False)
                            nc.tensor.matmul(psSc[:], yB[:, tsl], yB2[:, c],
                                             start=False, stop=False)
                            nc.tensor.matmul(psSc[:], yB2[:, tsl], yB[:, c],
                                             start=False, stop=False)
                        nc.tensor.matmul(psSc[:], onesNx[:], nxB[:, c],
                                         start=False, stop=True)
                        nc.scalar.activation(S[:, nj * 512:(nj + 1) * 512],
                                             psSc[:], AF.Copy)
                    st[t] = {"S": S}

                def topk_phase(t):
                    S = st[t]["S"]
                    # embed the column index in the low 11 mantissa bits:
                    # S_enc = (S & ~0x7FF) | j. Rank order is preserved up to
                    # a 2^-12 relative quantization; ties become impossible
                    # and indices are recovered by masking the winning
                    # values, so no find_index8 passes are needed.
                    nc.vector.scalar_tensor_tensor(
                        out=S[:].bitcast(u32), in0=S[:].bitcast(u32),
                        scalar=maskHi[:, 0:1], in1=iota2048[:],
                        op0=AOT.bitwise_and, op1=AOT.bitwise_or)
                    # two-stage top-k: per-256-column-chunk top-8 (8 cheap
                    # max8 passes), then top-24 of the 64 survivors. Exact
                    # for ranks 1-8 always; ranks 9-20 are exact unless one
                    # chunk holds >8 of the row's true top-20 (P ~ 3e-3 per
                    # row for index-uniform neighbor sets; the miss swaps a
                    # late-rank neighbor for the next-nearest — negligible).
                    maxv8 = small.tile([128, 64], f32, tag="maxv8")
                    for c in range(8):
                        nc.vector.max(maxv8[:, 8 * c:8 * (c + 1)],
                                      S[:, 256 * c:256 * (c + 1)])
                        if c == 3:
                            # PE keep-alive mid-stretch (HAM clock gate)
                            psJ = pyp.tile([8, 8], f32, tag="pyp")
                            nc.tensor.matmul(psJ[:], maxv8[0:1, 0:8],
                                             maxv8[0:1, 0:8],
                                             start=True, stop=True,
                                             skip_group_check=True)
                    maxv = small.tile([128, 24], f32, tag="maxv")
                    idxu = small.tile([128, 24], u32, tag="idxu")
                    for r in range(3):
                        rs = slice(8 * r, 8 * (r + 1))
                        nc.vector.max(maxv[:, rs], maxv8[:])
                        if r == 1:
                            # light PE keep-alive through the merge stretch
                            psJ = pyp.tile([8, 8], f32, tag="pyp")
                            nc.tensor.matmul(psJ[:], maxv[0:1, rs],
                                             maxv[0:1, rs],
                                             start=True, stop=True,
                                             skip_group_check=True)
                        if r < 2:
                            nc.vector.match_replace(maxv8[:], maxv[:, rs],
                                                    maxv8[:], -1e30)
                    nc.vector.tensor_scalar(out=idxu[:], in0=maxv[:].bitcast(u32),
                                            scalar1=maskLo[:, 0:1], scalar2=None,
                                            op0=AOT.bitwise_and)
                    st[t]["idxu"] = idxu

                def plumb_phase(t):
                    # index plumbing [128,20] u32 -> wrapped [128,160] i16.
                    # Emitted one iteration after topk(t) so the transposes
                    # never head-of-line-block the PE queue. The whole chain
                    # runs at high priority: it feeds the gathers, which pace
                    # the dst pipeline — without this the small idx copies
                    # queue behind bulk ACT work and stretch the loop.
                    ctx2 = tc.high_priority()
                    ctx2.__enter__()
                    idxu = st[t]["idxu"]
                    idxf = small.tile([128, 20], f32, tag="idxf")
                    nc.vector.tensor_copy(idxf[:], idxu[:, 0:20])
                    psT1 = pmix.tile([20, 128], f32, tag="pmix")
                    nc.tensor.transpose(psT1[:], idxf[:], ident[:])
                    idxT = small.tile([20, 128], f32, tag="idxT")
                    nc.scalar.activation(idxT[:], psT1[:], AF.Copy)
                    psT2 = pmix.tile([16, 8, 20], f32, tag="pmix")
                    for pg in range(8):
                        nc.tensor.transpose(psT2[:, pg, :],
                                            idxT[:, pg * 16:(pg + 1) * 16],
                                            ident[0:20, 0:20])
                    # replicate the 16 index rows to all 128 partitions with
                    # one tiny matmul (rep16 0/1 lhsT) instead of 3 chained
                    # SBUF->SBUF DMAs — shorter critical path to the gathers.
                    idxh = small.tile([16, 160], f16, tag="idxh")
                    nc.scalar.activation(
                        idxh[:].rearrange("q (c pg) -> q pg c", pg=8),
                        psT2[:], AF.Copy)
                    psRep = pmix.tile([128, 160], f32, tag="pmix")
                    nc.tensor.matmul(psRep[:], rep16[:], idxh[:],
                                     start=True, stop=True)
                    idxs16 = small.tile([128, 160], i16, tag="idxs16")
                    nc.scalar.activation(idxs16[:], psRep[:], AF.Copy)
                    dst = dstp.tile([128, K, CO2], gdt, tag="dst", name=f"dst{li}_{t}")
                    for qi, (off, n) in enumerate(GATHER_SPLITS):
                        nc.gpsimd.dma_gather(
                            dst[:, off // 128:(off + n) // 128, :], vtab[:],
                            idxs16[:, off // 16:(off + n) // 16], n, n, CO2,
                            queue_num=qi)
                    ctx2.__exit__(None, None, None)
                    st[t]["dst"] = dst

                def back(t):
                    dst = st.pop(t)["dst"]
                    tsl = slice(t * 128, (t + 1) * 128)
                    if last_layer:
                        # L3: plain-v bf16 rows; dsq via ACT Square.
                        dbf = dst
                        dsq = allp.tile([128, K, CO], bf16, tag="dsq",
                                        name=f"dsq{li}_{t}")
                        nc.scalar.activation(dsq[:], dst[:], AF.Square)
                        vsl = slice(0, CO)
                    elif li == 1:
                        # L2: bf16 [v|v^2] rows used directly.
                        dbf = dst
                        dsq = None
                        vsl = slice(0, CO)
                    else:
                        # L1: f32 [v|v^2] rows -> one bf16 copy of both halves
                        dbf = bfp.tile([128, K, CO2], bf16, tag="dbf")
                        nc.scalar.activation(dbf[:], dst[:], AF.Copy)
                        dsq = None
                        vsl = slice(0, CO)
                    if gdt is bf16:
                        # m = max_j v: 2x bf16 max tree 20->10->5->(2,2,1)->1
                        mt1 = bfp.tile([128, 10, CO], bf16, tag="mt1")
                        nc.vector.tensor_tensor(out=mt1[:], in0=dst[:, 0:10, vsl],
                                                in1=dst[:, 10:20, vsl], op=AOT.max)
                        mt2 = vcp.tile([128, 5, CO], bf16, tag="mt2")
                        nc.vector.tensor_tensor(out=mt2[:], in0=mt1[:, 0:5, :],
                                                in1=mt1[:, 5:10, :], op=AOT.max)
                        nc.vector.tensor_tensor(out=mt2[:, 0:2, :],
                                                in0=mt2[:, 0:2, :],
                                                in1=mt2[:, 2:4, :], op=AOT.max)
                        nc.vector.tensor_tensor(out=mt2[:, 0:1, :],
                                                in0=mt2[:, 0:1, :],
                                                in1=mt2[:, 1:2, :], op=AOT.max)
                        mloc = vcp.tile([128, CO], f32, tag="mloc")
                        nc.vector.tensor_tensor(out=mloc[:],
                                                in0=mt2[:, 0:1, :],
                                                in1=mt2[:, 4:5, :], op=AOT.max)
                    else:
                        mloc = vcp.tile([128, CO], f32, tag="mloc")
                        nc.vector.tensor_reduce(
                            mloc[:], dst[:, :, vsl].rearrange("p j c -> p c j"),
                            mybir.AxisListType.X, AOT.max)
                    # stats matmuls (bf16). For sq_tab layers one jgroup
                    # matmul covers [v|v^2] together (psS row folds to
                    # [Sum v | Sum v^2]); L3 keeps separate psS/psQ passes.
                    first = (t == 0)
                    last = (t == NT - 1)
                    for gi, (j0, gn) in enumerate(jgroups):
                        w = gn * CO2
                        nc.tensor.matmul(psS[:, 0:w], onesCol_bf[:],
                                         dbf[:, j0:j0 + gn, :],
                                         start=(first and gi == 0),
                                         stop=(last and gi == len(jgroups) - 1),
                                         skip_group_check=True)
                    if last_layer:
                        for gi, (j0, gn) in enumerate(jgroups):
                            w = gn * CO2
                            nc.tensor.matmul(psQ[:, 0:w], onesCol_bf[:],
                                             dsq[:, j0:j0 + gn, :],
                                             start=(first and gi == 0),
                                             stop=(last and gi == len(jgroups) - 1),
                                             skip_group_check=True)
                    for h in range(NH):
                        for gi, (j0, gn) in enumerate(jgroups):
                            w = gn * CO2
                            nc.tensor.matmul(
                                psX[h][:, 0:w],
                                ubf_all[:, t, 128 * h:128 * h + CH],
                                dbf[:, j0:j0 + gn, :],
                                start=(first and gi == 0),
                                stop=(last and gi == len(jgroups) - 1),
                                skip_group_check=True)
                    # pre-barrier y: wsum, transpose, stage into yPre / gmax
                    wsum = vcp.tile([128, CO], f32, tag="wsum")
                    nc.vector.tensor_tensor(out=wsum[:], in0=u_all[:, t, :],
                                            in1=mloc[:], op=AOT.add)
                    for h in range(NH):
                        psY = pyp.tile([128, 128], f32, tag="pyp")
                        nc.tensor.transpose(psY[0:CH, :],
                                            wsum[:, 128 * h:128 * h + CH],
                                            ident[:])
                        if not last_layer:
                            nc.scalar.activation(yPre[:, tsl], psY[0:CH, :],
                                                 AF.Copy)
                        else:
                            nc.vector.tensor_tensor(out=gmax[:, h, :],
                                                    in0=gmax[:, h, :],
                                                    in1=psY[0:CH, :], op=AOT.max)

                # tiles 0/1 scores+topk first so the DVE starts immediately;
                # u/v table prep then overlaps their topk on PE/ACT, and
                # plumb(0) lands in the PE queue right after prep (by which
                # time topk(0) has finished -> no head-of-line stall).
                scores_phase(0)
                topk_phase(0)
                scores_phase(1)
                topk_phase(1)

                # u|v computed together per tile: psUV = yB^T[wuv] via the
                # same 3-term bf16 scheme (hi*hi + hi*lo + lo*hi) + bias rows.
                uv_hi, uv_lo = yB[0:CI, :], yB2[0:CI, :]
                for t in range(NT):
                    tsl = slice(t * 128, (t + 1) * 128)
                    psUV = pscore.tile([128, 512], f32, tag="psc",
                                       name=f"psUV{li}_{t}")
                    W = 2 * CO
                    nc.tensor.matmul(psUV[:, 0:W], uv_hi[:, tsl], wuvh[:],
                                     start=True, stop=False)
                    nc.tensor.matmul(psUV[:, 0:W], uv_hi[:, tsl], wuvl[:],
                                     start=False, stop=False)
                    nc.tensor.matmul(psUV[:, 0:W], uv_lo[:, tsl], wuvh[:],
                                     start=False, stop=False)
                    nc.tensor.matmul(psUV[:, 0:W], ones2[:], brw[:],
                                     start=False, stop=True)
                    nc.scalar.activation(u_all[:, t, :], psUV[:, 0:CO], AF.Copy)
                    nc.scalar.activation(ubf_all[:, t, :], psUV[:, 0:CO], AF.Copy)
                    vst = vcp.tile([128, CO2], gdt, tag="vst")
                    nc.scalar.activation(vst[:, 0:CO], psUV[:, CO:W], AF.Copy)
                    if sq_tab:
                        nc.scalar.activation(vst[:, CO:CO2], psUV[:, CO:W],
                                             AF.Square)
                    nc.sync.dma_start(vtab[tsl, :], vst[:])

                # Sum u / Sum u^2 (fp32, exact)
                u_flat = u_all[:].rearrange("p t c -> p (t c)")
                nuv = NT * CO // 512
                for s in range(nuv):
                    usqf = vcp.tile([128, 512], f32, tag="usq")
                    nc.scalar.activation(usqf[:], u_flat[:, 512 * s:512 * (s + 1)],
                                         AF.Square)
                    nc.tensor.matmul(psSu, onesColF[:],
                                     u_flat[:, 512 * s:512 * (s + 1)],
                                     start=(s == 0), stop=(s == nuv - 1),
                                     skip_group_check=True)
                    nc.tensor.matmul(psQu, onesColF[:], usqf[:],
                                     start=(s == 0), stop=(s == nuv - 1),
                                     skip_group_check=True)
                rowSu = rows.tile([1, 512], f32, tag="rowSu")
                nc.scalar.activation(rowSu[:], psSu, AF.Copy)
                rowQu = rows.tile([1, 512], f32, tag="rowQu")
                nc.scalar.activation(rowQu[:], psQu, AF.Copy)
                for row in (rowSu, rowQu):
                    wfull = 512
                    while wfull > CO:
                        half = wfull // 2
                        nc.vector.tensor_tensor(out=row[:, 0:half],
                                                in0=row[:, 0:half],
                                                in1=row[:, half:wfull], op=AOT.add)
                        wfull = half

                psSQ = pstat.tile([33, 512], f32, tag="psSQ", name="psSQm")
                psS = psSQ[0:1, :]
                psQ = psSQ[32:33, :]

                plumb_phase(0)

                # 3-deep pipeline (dst triple-buffered): gathers run two
                # tiles ahead of the stats consumer, so the gather-DMA /
                # back() WAR loop no longer serializes tile pairs.
                # L1 additionally nudges back(t) later on the scheduler's
                # cost-model timeline: the model underestimates the gather
                # DMA, so without the nudge it orders dst-consumers (dbf,
                # mloc, stats) BEFORE the next tiles' idx-plumb ops in each
                # engine queue, and on HW they stall head-of-line on the
                # gather tail, stretching the loop to ~19us/tile.
                for it in range(2, NT + 3):
                    if it - 1 < NT:
                        plumb_phase(it - 1)
                    if it < NT:
                        scores_phase(it)
                    if li == 0 and it >= 3:
                        with tc.tile_wait_until(0.025 + (it - 3) * 0.011):
                            back(it - 3)
                    elif li == 1 and it >= 3:
                        # conservative-low base: too-low times are no-ops in
                        # the sim, too-high would phase-separate the layer
                        with tc.tile_wait_until(0.225 + (it - 3) * 0.011):
                            back(it - 3)
                    elif it >= 3:
                        back(it - 3)
                    if it < NT:
                        topk_phase(it)

                # ---------- copy out S/Q, then fold 512 -> CO2 ----------
                # sq_tab layers: psS already holds [Sum v | Sum v^2] blocks.
                rowS = rows.tile([1, 512], f32, tag="rowS")
                nc.scalar.activation(rowS[:], psS, AF.Copy)
                if last_layer:
                    rowQ = rows.tile([1, 512], f32, tag="rowQ")
                    nc.scalar.activation(rowQ[:], psQ, AF.Copy)
                    foldrows = (rowS, rowQ)
                else:
                    foldrows = (rowS,)
                for row in foldrows:
                    wfull = 512
                    while wfull > CO2:
                        half = wfull // 2
                        nc.vector.tensor_tensor(out=row[:, 0:half],
                                                in0=row[:, 0:half],
                                                in1=row[:, half:wfull], op=AOT.add)
                        wfull = half
                if not last_layer:
                    rowQ = rowS[:, CO:2 * CO]

                # cross-term: diag of psX via ttr with mask, then -> row
                junk = small.tile([128, 512], f32, tag="junk")
                crossRow = rows.tile([1, 256], f32, tag="crossRow")
                for h in range(NH):
                    ccol = small.tile([128, 1], f32, tag="ccol")
                    nc.vector.tensor_tensor(out=junk[0:CH, :], in0=psX[h][:],
                                            in1=masks[h][:], op=AOT.mult)
                    nc.vector.tensor_reduce(ccol[0:CH, :], junk[0:CH, :],
                                            mybir.AxisListType.X, AOT.add)
                    psCr = pmix.tile([1, CH], f32, tag="pmix")
                    nc.tensor.transpose(psCr[:], ccol[0:CH, :], ident[0:CH, 0:CH])
                    nc.scalar.activation(crossRow[:, 128 * h:128 * h + CH],
                                         psCr[:], AF.Copy)

                # ---------- per-core partial sums -> allreduce ----------
                statsrow = rows.tile([1, 512], f32, tag="statsrow")
                nc.vector.tensor_scalar(out=statsrow[:, 0:CO], in0=rowSu[:, 0:CO],
                                        scalar1=float(K), scalar2=None,
                                        op0=AOT.mult)
                nc.vector.tensor_tensor(out=statsrow[:, 0:CO],
                                        in0=statsrow[:, 0:CO],
                                        in1=rowS[:, 0:CO], op=AOT.add)
                nc.vector.tensor_scalar(out=statsrow[:, CO:2 * CO],
                                        in0=rowQu[:, 0:CO], scalar1=float(K),
                                        scalar2=None, op0=AOT.mult)
                nc.vector.tensor_scalar(out=crossRow[:, 0:CO], in0=crossRow[:, 0:CO],
                                        scalar1=2.0, scalar2=None, op0=AOT.mult)
                nc.vector.tensor_tensor(out=statsrow[:, CO:2 * CO],
                                        in0=statsrow[:, CO:2 * CO],
                                        in1=crossRow[:, 0:CO], op=AOT.add)
                nc.vector.tensor_tensor(out=statsrow[:, CO:2 * CO],
                                        in0=statsrow[:, CO:2 * CO],
                                        in1=rowQ[:, 0:CO], op=AOT.add)

                ccin = dram.tile([1, 2 * CO], f32, tag=f"ccin{li}")
                ccout = dram.tile([1, 2 * CO], f32, tag=f"ccout{li}")
                nc.sync.dma_start(ccin[:], statsrow[:, 0:2 * CO])
                nc.gpsimd.collective_compute(
                    "AllReduce", AOT.add,
                    replica_groups=[list(range(NCORES))],
                    ins=[ccin.opt()], outs=[ccout.opt()])
                statsg = rows.tile([1, 512], f32, tag="statsg")
                nc.sync.dma_start(statsg[:, 0:2 * CO], ccout[:])

                # ---------- BN scale/shift ----------
                cntr = 1.0 / float(B * N * K)
                meanR = rows.tile([1, 256], f32, tag="meanR")
                nc.vector.tensor_scalar(out=meanR[:, 0:CO], in0=statsg[:, 0:CO],
                                        scalar1=cntr, scalar2=None, op0=AOT.mult)
                t1R = rows.tile([1, 256], f32, tag="t1R")
                t2R = rows.tile([1, 256], f32, tag="t2R")
                nc.vector.tensor_scalar(out=t1R[:, 0:CO], in0=statsg[:, CO:2 * CO],
                                        scalar1=cntr, scalar2=None, op0=AOT.mult)
                nc.vector.tensor_tensor(out=t2R[:, 0:CO], in0=meanR[:, 0:CO],
                                        in1=meanR[:, 0:CO], op=AOT.mult)
                nc.vector.tensor_tensor(out=t1R[:, 0:CO], in0=t1R[:, 0:CO],
                                        in1=t2R[:, 0:CO], op=AOT.subtract)
                nc.vector.tensor_scalar(out=t1R[:, 0:CO], in0=t1R[:, 0:CO],
                                        scalar1=1e-5, scalar2=None, op0=AOT.add)
                nc.scalar.activation(t2R[:, 0:CO], t1R[:, 0:CO], AF.Sqrt)
                nc.vector.reciprocal(t1R[:, 0:CO], t2R[:, 0:CO])
                scaleR = rows.tile([1, 256], f32, tag="scaleR")
                nc.vector.tensor_tensor(out=scaleR[:, 0:CO], in0=grow[:].bitcast(f32),
                                        in1=t1R[:, 0:CO], op=AOT.mult)
                shiftR = rows.tile([1, 256], f32, tag="shiftR")
                nc.vector.tensor_tensor(out=shiftR[:, 0:CO], in0=meanR[:, 0:CO],
                                        in1=scaleR[:, 0:CO], op=AOT.mult)
                nc.vector.tensor_tensor(out=shiftR[:, 0:CO], in0=berow[:],
                                        in1=shiftR[:, 0:CO], op=AOT.subtract)

                scol = allp.tile([128, 2], f32, tag="scol")
                tcol = allp.tile([128, 2], f32, tag="tcol")
                for h in range(NH):
                    psc = pmix.tile([128, 1], f32, tag="pmix")
                    nc.tensor.transpose(psc[0:CH, :],
                                        scaleR[:, 128 * h:128 * h + CH],
                                        ident[0:1, 0:1])
                    nc.scalar.activation(scol[0:CH, h:h + 1], psc[0:CH, :], AF.Copy)
                    psc2 = pmix.tile([128, 1], f32, tag="pmix")
                    nc.tensor.transpose(psc2[0:CH, :],
                                        shiftR[:, 128 * h:128 * h + CH],
                                        ident[0:1, 0:1])
                    nc.scalar.activation(tcol[0:CH, h:h + 1], psc2[0:CH, :], AF.Copy)

                # ---------- y-phase (post-barrier): single affine+relu ----------
                if not last_layer:
                    # safe to reuse the single yT buffer: yT's last readers
                    # are this layer's prep (hi/lo split + ysq), long retired
                    # by the time the post-barrier y-phase writes.
                    yTn = ytp.tile([128, N], f32, tag="yt0")
                    nc.scalar.activation(yTn[0:CH, :], yPre[:, :], AF.Relu,
                                         bias=tcol[0:CH, 0:1],
                                         scale=scol[0:CH, 0:1])
                    yT = yTn

            # ---------- head ----------
            psH = pmix.tile([1, 256], f32, tag="pmix")
            for h in range(2):
                gcol = small.tile([128, 1], f32, tag="ccol")
                nc.vector.tensor_reduce(gcol[:], gmax[:, h, :],
                                        mybir.AxisListType.X, AOT.max)
                nc.vector.tensor_scalar(out=gcol[:], in0=gcol[:],
                                        scalar1=scol[:, h:h + 1],
                                        scalar2=tcol[:, h:h + 1],
                                        op0=AOT.mult, op1=AOT.add)
                nc.vector.tensor_scalar_max(gcol[:], gcol[:], 0.0)
                nc.tensor.matmul(psH[:], gcol[:], woT_sb[:, h, :],
                                 start=(h == 0), stop=False,
                                 skip_group_check=True)
            nc.tensor.matmul(psH[:], onesRow[:, 0:1], boRow[:],
                             start=False, stop=True, skip_group_check=True)
            outSb = rows.tile([1, 256], f32, tag="crossRow")
            nc.scalar.activation(outSb[:], psH[:], AF.Copy)
            nc.sync.dma_start(out_ext[:], outSb[:])

    nc.compile()
    return nc


def _host_prep(x, weights):
    """Build per-core input maps. x: [B, N, 3]."""
    import ml_dtypes
    bf = ml_dtypes.bfloat16
    shared = {}
    for li, (ci, co) in enumerate(LAYERS):
        W = np.asarray(weights[f"w{li + 1}"])            # [co, 2*ci]
        wc, wnn = W[:, :ci], W[:, ci:]
        wuv = np.concatenate([(wc - wnn).T, wnn.T], axis=1).astype(np.float32)
        hi = wuv.astype(bf)
        lo = (wuv - hi.astype(np.float32)).astype(bf)
        shared[f"wuvh{li}"] = np.ascontiguousarray(hi)
        shared[f"wuvl{li}"] = np.ascontiguousarray(lo)
        bias = np.asarray(weights[f"b{li + 1}"]).astype(np.float32).reshape(co)
        brw = np.zeros((2, 2 * co), np.float32)
        bhi = bias.astype(bf).astype(np.float32)
        brw[0, :co] = bhi
        brw[1, :co] = bias - bhi
        shared[f"brw{li}"] = brw.astype(bf)
        shared[f"grow{li}"] = np.asarray(weights[f"g{li + 1}"]).reshape(1, co).astype(np.float32)
        shared[f"berow{li}"] = np.asarray(weights[f"be{li + 1}"]).reshape(1, co).astype(np.float32)
        # psX diag mask: v-diagonal within each jgroup block. sq_tab layers
        # (L1/L2) gather [v|v^2] so the block is 2*co wide with the v part
        # first; L3 keeps plain co-wide blocks.
        blk = co if li == len(LAYERS) - 1 else 2 * co
        Gm = 512 // blk
        for h in range(-(-co // 128)):
            hc = min(128, co - 128 * h)
            mk = np.zeros((hc, 512), np.float32)
            for p in range(hc):
                for j in range(Gm):
                    mk[p, j * blk + p + 128 * h] = 1.0
            shared[f"mask{li}_{h}"] = mk
    shared["ident"] = np.eye(128, dtype=np.float32)
    rep = np.zeros((16, 128), np.float16)
    for r in range(16):
        rep[r, r::16] = 1.0
    shared["rep16"] = rep
    shared["woT"] = np.ascontiguousarray(np.asarray(weights["wo"]).T.astype(np.float32))
    shared["boRow"] = np.asarray(weights["bo"]).reshape(1, 256).astype(np.float32)
    ins = []
    for c in range(NCORES):
        m = dict(shared)
        m["xT"] = np.ascontiguousarray(np.asarray(x[c]).T.astype(np.float32))
        ins.append(m)
    return ins


def kernel(**inputs):
    from concourse.bass_utils import run_bass_kernel_spmd
    x = np.asarray(inputs["x"])
    if "nc" not in _BUILT:
        _BUILT["nc"] = _build()
    nc = _BUILT["nc"]
    in_maps = _host_prep(x, inputs)
    res = run_bass_kernel_spmd(nc, in_maps, list(range(NCORES))).results
    out = np.stack([res[c]["out"][0] for c in range(NCORES)], axis=0)
    return out.astype(np.float32)



# revision 58
# speedup vs baseline: 1.0329x; 1.0329x over previous
"""DynamicGraphCNN (DGCNN) forward pass on 8 Trainium2 NeuronCores.

Data-parallel over batch B=8: one point cloud per core. Per layer (edge-conv):
  scores  S'[i,j] = <x_i, x_j> - ||x_j||^2/2    (rank-equivalent to -dist^2)
  top-20 neighbors per row via DVE max/max_index/match_replace
  h[i,j] = u_i + v_{n(i,j)} with u = x(Wc-Wn)^T + b, v = x Wn^T
  BN (training stats over B,N,k) from global sums:
      Sum h   = 20*Sum u + Sum_{ij} v_n
      Sum h^2 = 20*Sum u^2 + 2*Sum_i u_i.s_i + Sum_{ij} v_n^2
  computed with bf16 PE matmuls over the gathered tiles (j-packed into psum),
  cross-term via u-weighted matmuls + diagonal-mask extraction.
  Cross-core reduction: one 8-core AllReduce per layer.
  y_i = relu(scale*(u_i + max_j v_n) + shift)   (monotone: max before affine)
Final: global max over points, then linear head.

v2: negxx fused into the score matmul via an augmented lhs row (L1/L2),
4-way SWDGE queue spread for the gathers, bf16 gather tables for L2/L3
(halving gather DMA and enabling 2x bf16 max-trees), stats matmuls grouped
by stationary operand to reuse loaded weights.
"""
import sys
sys.path.insert(0, '/opt/trn_rl_repo')

import numpy as np

B, N, K = 8, 2048, 20
NT = N // 128                      # 16 point tiles of 128
LAYERS = [(3, 64), (64, 128), (128, 256)]
NCORES = 8
GATHER_SPLITS = [(0, 640), (640, 640), (1280, 640), (1920, 640)]

_BUILT = {}


def _build(dbg=False):
    import contextlib
    import concourse.bacc as bacc
    import concourse.mybir as mybir
    import concourse.tile as tile

    f32 = mybir.dt.float32
    f32r = mybir.dt.float32r
    bf16 = mybir.dt.bfloat16
    i16 = mybir.dt.int16
    f16 = mybir.dt.float16
    u32 = mybir.dt.uint32
    AOT = mybir.AluOpType
    AF = mybir.ActivationFunctionType

    nc = bacc.Bacc("TRN2", target_bir_lowering=False, debug=False,
                   num_devices=NCORES, num_swdge_queues=4)

    # ---------------- external tensors ----------------
    xT_in = nc.dram_tensor("xT", [3, N], f32, kind="ExternalInput")
    ext = {}
    for li, (ci, co) in enumerate(LAYERS):
        # wuv = [wcm | wn] (u and v weights side by side), pre-split hi/lo
        # bf16 on the host for the 3-term bf16 matmuls; brw = [bias_hi;
        # bias_lo] rows (v half zero).
        ext[f"wuvh{li}"] = nc.dram_tensor(f"wuvh{li}", [ci, 2 * co], bf16, kind="ExternalInput")
        ext[f"wuvl{li}"] = nc.dram_tensor(f"wuvl{li}", [ci, 2 * co], bf16, kind="ExternalInput")
        ext[f"brw{li}"] = nc.dram_tensor(f"brw{li}", [2, 2 * co], bf16, kind="ExternalInput")
        for rn in ("grow", "berow"):
            ext[f"{rn}{li}"] = nc.dram_tensor(f"{rn}{li}", [1, co], f32, kind="ExternalInput")
        for h in range(-(-co // 128)):
            hc = min(128, co - 128 * h)
            ext[f"mask{li}_{h}"] = nc.dram_tensor(
                f"mask{li}_{h}", [hc, 512], f32, kind="ExternalInput")
    ident_in = nc.dram_tensor("ident", [128, 128], f32, kind="ExternalInput")
    rep16_in = nc.dram_tensor("rep16", [16, 128], mybir.dt.float16, kind="ExternalInput")
    woT_in = nc.dram_tensor("woT", [256, 256], f32, kind="ExternalInput")
    bo_in = nc.dram_tensor("boRow", [1, 256], f32, kind="ExternalInput")
    out_ext = nc.dram_tensor("out", [1, 256], f32, kind="ExternalOutput")

    with tile.TileContext(nc) as tc:
        ctx = contextlib.ExitStack()
        with ctx:
            big = ctx.enter_context(tc.tile_pool(name="big", bufs=3))      # S / ysq
            ytp = ctx.enter_context(tc.tile_pool(name="ytp", bufs=1))      # yT (2 tags)
            allp = ctx.enter_context(tc.tile_pool(name="allp", bufs=1))    # layer residents
            resid = ctx.enter_context(tc.tile_pool(name="resid", bufs=1))  # constants
            dstp = ctx.enter_context(tc.tile_pool(name="dstp", bufs=3))
            bfp = ctx.enter_context(tc.tile_pool(name="bfp", bufs=2))      # dsq / trees
            small = ctx.enter_context(tc.tile_pool(name="small", bufs=2))  # idx plumbing
            rows = ctx.enter_context(tc.tile_pool(name="rows", bufs=1))    # [1,*] rows
            vcp = ctx.enter_context(tc.tile_pool(name="vcp", bufs=2))      # staging
            dram = ctx.enter_context(tc.tile_pool(name="dram", bufs=1, space="DRAM"))
            pscore = ctx.enter_context(tc.tile_pool(name="pscore", bufs=2, space="PSUM"))
            pyp = ctx.enter_context(tc.tile_pool(name="pyp", bufs=1, space="PSUM"))
            pmix = ctx.enter_context(tc.tile_pool(name="pmix", bufs=2, space="PSUM"))
            pstat = ctx.enter_context(tc.tile_pool(name="pstat", bufs=1, space="PSUM"))

            # ---------- kernel-lifetime constants ----------
            ident = resid.tile([128, 128], f32, tag="ident")
            nc.sync.dma_start(ident[:], ident_in[:])
            rep16 = resid.tile([16, 128], f16, tag="rep16")
            nc.sync.dma_start(rep16[:], rep16_in[:])
            onesRow = resid.tile([1, 128], f32, tag="onesRow")
            nc.vector.memset(onesRow[:], 1.0)
            ones2 = resid.tile([2, 128], bf16, tag="ones2")
            nc.vector.memset(ones2[:], 1.0)
            # picks rows 0 (negxx hi) and 32 (negxx lo) out of nxB
            onesNx = resid.tile([33, 128], bf16, tag="onesNx")
            nc.vector.memset(onesNx[:], 0.0)
            nc.vector.memset(onesNx[0:1, :], 1.0)
            nc.vector.memset(onesNx[32:33, :], 1.0)
            onesColF = resid.tile([128, 1], f32, tag="onesColF")
            nc.vector.memset(onesColF[:], 1.0)
            onesCol_bf = resid.tile([128, 1], bf16, tag="onesColbf")
            nc.vector.memset(onesCol_bf[:], 1.0)
            negHalfCol = resid.tile([128, 1], f32, tag="negHalfCol")
            nc.vector.memset(negHalfCol[:], -0.5)
            woT_sb = resid.tile([128, 2, 256], f32, tag="woT")
            for h in range(2):
                nc.sync.dma_start(woT_sb[:, h, :], woT_in[128 * h:128 * (h + 1), :])
            boRow = resid.tile([1, 256], f32, tag="boRow")
            nc.sync.dma_start(boRow[:], bo_in[:])
            gmax = resid.tile([128, 2, 128], f32, tag="gmax")
            nc.vector.memset(gmax[:], -1e30)
            # per-partition column-index row 0..2047, used to embed j into the
            # low 11 mantissa bits of the scores (index recovery without
            # find_index8 passes)
            iota2048 = resid.tile([128, N], u32, tag="iota2048")
            nc.gpsimd.iota(iota2048[:], pattern=[[1, N]], base=0,
                           channel_multiplier=0)
            maskHi = resid.tile([128, 1], u32, tag="maskHi")
            nc.vector.memset(maskHi[:], 0xFFFFF800)
            maskLo = resid.tile([128, 1], u32, tag="maskLo")
            nc.vector.memset(maskLo[:], 0x7FF)

            # yT carries the CI feature rows plus one negxx row for L1/L2 so
            # the -||x_j||^2/2 bias rides the score matmul as an extra
            # contraction row. Engine accesses need 32-aligned base
            # partitions, so L1 (CI=3) pads rows 3..31 with zeros and puts
            # negxx at row 32; L2 (CI=64) puts it at row 64. L3 (CI=128) has
            # no spare partition and keeps the separate bias matmul.
            yT = ytp.tile([128, N], f32, tag="yt0")
            nc.vector.memset(yT[0:33, :], 0.0)
            nc.sync.dma_start(yT[0:3, :], xT_in[:])

            for li, (CI, CO) in enumerate(LAYERS):
                NH = -(-CO // 128)
                CH = min(128, CO)
                last_layer = (li == len(LAYERS) - 1)
                # L1/L2 gather [v | v^2] rows (sq_tab): doubles the row to
                # 512B (full DMA line rate vs the <512B half-rate penalty,
                # so same gather time) and kills the per-tile dsq Square.
                # L3 keeps plain v rows (doubling them would double an
                # already-full-rate 512B gather).
                sq_tab = not last_layer
                CO2 = 2 * CO if sq_tab else CO
                # L1 keeps exact f32 v (and 2*64*4 = 512B full-rate rows);
                # L2/L3 use bf16 tables (512B rows for L2's [v|v^2]).
                gdt = f32 if li == 0 else bf16
                G = 512 // CO2
                jgroups = []
                j0 = 0
                while j0 < K:
                    jgroups.append((j0, min(G, K - j0)))
                    j0 += G

                # ---------- weights / rows ----------
                wuvh = allp.tile([CI, 2 * CO], bf16, tag="wuvh")
                nc.sync.dma_start(wuvh[:], ext[f"wuvh{li}"][:])
                wuvl = allp.tile([CI, 2 * CO], bf16, tag="wuvl")
                nc.sync.dma_start(wuvl[:], ext[f"wuvl{li}"][:])
                brw = allp.tile([2, 2 * CO], bf16, tag="brw")
                nc.sync.dma_start(brw[:], ext[f"brw{li}"][:])
                grow = allp.tile([1, CO], f32, tag="grow")
                nc.sync.dma_start(grow[:], ext[f"grow{li}"][:])
                berow = allp.tile([1, CO], f32, tag="berow")
                nc.sync.dma_start(berow[:], ext[f"berow{li}"][:])
                masks = []
                for h in range(NH):
                    mk = allp.tile([CH, 512], f32, tag=f"mask{h}")
                    nc.sync.dma_start(mk[:], ext[f"mask{li}_{h}"][:])
                    masks.append(mk)

                # ---------- prep: bf16 hi/lo score operands + negxx ----------
                # Scores run as 3-term bf16 matmuls (hi*hi + hi*lo + lo*hi,
                # dropping lo*lo ~ 2^-18): bf16 streams at 1 cyc/row on PE
                # vs fp32's 4, and ranking error stays far below the 2^-12
                # index-embedding quantization. negxx is computed exactly in
                # fp32 then split hi/lo the same way. Layouts (32-aligned
                # base partitions for every engine access):
                #   L1: yB2 = [hi@0, lo@32, hi@64, nxHi@96, nxLo@97] (1 mm)
                #   L2: yB = [hi@0, lo@64]; yB2 = [hi@0, nxHi@64, nxLo@65]
                #   L3: yB = hi, yB2 = lo, nxB = [nxHi, nxLo]  (4 mms)
                ysq = big.tile([128, N], f32, tag="big")
                nc.scalar.activation(ysq[0:CI, :], yT[0:CI, :], AF.Square)
                nxF = allp.tile([1, N], f32, tag="nxF")
                for nj in range(4):
                    ps = pmix.tile([1, 512], f32, tag="pmix")
                    nc.tensor.matmul(ps[:], negHalfCol[0:CI, :],
                                     ysq[0:CI, nj * 512:(nj + 1) * 512],
                                     start=True, stop=True)
                    nc.scalar.activation(nxF[:, nj * 512:(nj + 1) * 512],
                                         ps[:], AF.Copy)
                yB = allp.tile([128, N], bf16, tag="yB")
                yB2 = allp.tile([128, N], bf16, tag="yB2")
                nxB = allp.tile([33, N], bf16, tag="nxB")
                if li == 0:
                    nc.vector.memset(yB[0:96, :], 0.0)
                hi_dst = yB[0:CI, :]
                nc.scalar.activation(hi_dst, yT[0:CI, :], AF.Copy)
                yLoF = big.tile([128, N], f32, tag="big", name=f"yLoF{li}")
                nc.vector.tensor_tensor(out=yLoF[0:CI, :], in0=yT[0:CI, :],
                                        in1=hi_dst, op=AOT.subtract)
                # lo always lands at base partition 0 in yB2 (matmul lhsT and
                # rhs must share base_partition), plus packed copies inside yB
                # where the single-matmul rhs layouts need them.
                nc.scalar.activation(yB2[0:CI, :], yLoF[0:CI, :], AF.Copy)
                if li == 0:
                    # packed rhs layout: lo at rows 32-34, hi again at 64-66
                    nc.scalar.activation(yB[32:35, :], yLoF[0:CI, :], AF.Copy)
                    nc.scalar.activation(yB[64:67, :], yT[0:CI, :], AF.Copy)
                elif li == 1:
                    nc.scalar.activation(yB[64:128, :], yLoF[0:CI, :], AF.Copy)
                nc.scalar.activation(nxB[0:1, :], nxF[:], AF.Copy)
                nxLoF = allp.tile([1, N], f32, tag="nxLoF")
                nc.vector.tensor_tensor(out=nxLoF[:], in0=nxF[:],
                                        in1=nxB[0:1, :], op=AOT.subtract)
                nc.scalar.activation(nxB[32:33, :], nxLoF[:], AF.Copy)

                # ---------- stat psums (locked for the layer) ----------
                psSQu = pstat.tile([33, 512], f32, tag="psSQ", name="psSQu")
                psSu = psSQu[0:1, :]
                psQu = psSQu[32:33, :]
                psX = [pstat.tile([CH, 512], f32, tag=f"psX{h}", name=f"psX{li}_{h}") for h in range(NH)]

                yPre = None
                if not last_layer:
                    yPre = big.tile([CH, N], f32, tag="big", name=f"yPre{li}")

                vtab = dram.tile([N, CO2], gdt, tag=f"vtab{li}")
                u_all = allp.tile([128, NT, CO], f32, tag="u_all")
                ubf_all = allp.tile([128, NT, CO], bf16, tag="ubf_all")

                st = {}   # per-tile tiles: S, maxv, idxu, dst

                def scores_phase(t):
                    tsl = slice(t * 128, (t + 1) * 128)
                    S = big.tile([128, N], f32, tag="big", name=f"S{li}_{t}")
                    if li == 0:
                        aug = small.tile([96, 128], bf16, tag="aug0",
                                         name=f"aug0_{t}")
                        if t < 2:
                            nc.vector.memset(aug[:], 0.0)
                        nc.scalar.activation(aug[0:3, :], yB[0:3, tsl], AF.Copy)
                        nc.scalar.activation(aug[32:35, :], yB[0:3, tsl], AF.Copy)
                        nc.scalar.activation(aug[64:67, :], yB[32:35, tsl], AF.Copy)
                    elif li == 1:
                        # [hi_i; hi_i] packed so one matmul covers hi*hi+hi*lo
                        aug = small.tile([128, 128], bf16, tag="aug1",
                                         name=f"aug1_{t}")
                        nc.scalar.activation(aug[0:64, :], yB[0:64, tsl], AF.Copy)
                        nc.scalar.activation(aug[64:128, :], yB[0:64, tsl], AF.Copy)
                    for nj in range(4):
                        c = slice(nj * 512, (nj + 1) * 512)
                        psSc = pscore.tile([128, 512], f32, tag="psc")
                        if li == 0:
                            nc.tensor.matmul(psSc[:], aug[:], yB[0:96, c],
                                             start=True, stop=False)
                        elif li == 1:
                            nc.tensor.matmul(psSc[:], aug[:], yB[:, c],
                                             start=True, stop=False)
                            nc.tensor.matmul(psSc[:], yB2[0:64, tsl], yB[0:64, c],
                                             start=False, stop=False)
                        else:
                            nc.tensor.matmul(psSc[:], yB[:, tsl], yB[:, c],
                                             start=True, stop=False)
                            nc.tensor.matmul(psSc[:], yB[:, tsl], yB2[:, c],
                                             start=False, stop=False)
                            nc.tensor.matmul(psSc[:], yB2[:, tsl], yB[:, c],
                                             start=False, stop=False)
                        nc.tensor.matmul(psSc[:], onesNx[:], nxB[:, c],
                                         start=False, stop=True)
                        nc.scalar.activation(S[:, nj * 512:(nj + 1) * 512],
                                             psSc[:], AF.Copy)
                    st[t] = {"S": S}

                def topk_phase(t):
                    S = st[t]["S"]
                    # embed the column index in the low 11 mantissa bits:
                    # S_enc = (S & ~0x7FF) | j. Rank order is preserved up to
                    # a 2^-12 relative quantization; ties become impossible
                    # and indices are recovered by masking the winning
                    # values, so no find_index8 passes are needed.
                    nc.vector.scalar_tensor_tensor(
                        out=S[:].bitcast(u32), in0=S[:].bitcast(u32),
                        scalar=maskHi[:, 0:1], in1=iota2048[:],
                        op0=AOT.bitwise_and, op1=AOT.bitwise_or)
                    # two-stage top-k: per-256-column-chunk top-8 (8 cheap
                    # max8 passes), then top-24 of the 64 survivors. Exact
                    # for ranks 1-8 always; ranks 9-20 are exact unless one
                    # chunk holds >8 of the row's true top-20 (P ~ 3e-3 per
                    # row for index-uniform neighbor sets; the miss swaps a
                    # late-rank neighbor for the next-nearest — negligible).
                    maxv8 = small.tile([128, 64], f32, tag="maxv8")
                    for c in range(8):
                        nc.vector.max(maxv8[:, 8 * c:8 * (c + 1)],
                                      S[:, 256 * c:256 * (c + 1)])
                        if c == 3:
                            # PE keep-alive mid-stretch (HAM clock gate)
                            psJ = pyp.tile([8, 8], f32, tag="pyp")
                            nc.tensor.matmul(psJ[:], maxv8[0:1, 0:8],
                                             maxv8[0:1, 0:8],
                                             start=True, stop=True,
                                             skip_group_check=True)
                    maxv = small.tile([128, 24], f32, tag="maxv")
                    idxu = small.tile([128, 24], u32, tag="idxu")
                    for r in range(3):
                        rs = slice(8 * r, 8 * (r + 1))
                        nc.vector.max(maxv[:, rs], maxv8[:])
                        if r == 1:
                            # light PE keep-alive through the merge stretch
                            psJ = pyp.tile([8, 8], f32, tag="pyp")
                            nc.tensor.matmul(psJ[:], maxv[0:1, rs],
                                             maxv[0:1, rs],
                                             start=True, stop=True,
                                             skip_group_check=True)
                        if r < 2:
                            nc.vector.match_replace(maxv8[:], maxv[:, rs],
                                                    maxv8[:], -1e30)
                    nc.vector.tensor_scalar(out=idxu[:], in0=maxv[:].bitcast(u32),
                                            scalar1=maskLo[:, 0:1], scalar2=None,
                                            op0=AOT.bitwise_and)
                    st[t]["idxu"] = idxu

                def plumb_phase(t):
                    # index plumbing [128,20] u32 -> wrapped [128,160] i16.
                    # Emitted one iteration after topk(t) so the transposes
                    # never head-of-line-block the PE queue. The whole chain
                    # runs at high priority: it feeds the gathers, which pace
                    # the dst pipeline — without this the small idx copies
                    # queue behind bulk ACT work and stretch the loop.
                    ctx2 = tc.high_priority()
                    ctx2.__enter__()
                    idxu = st[t]["idxu"]
                    idxf = small.tile([128, 20], f32, tag="idxf")
                    nc.vector.tensor_copy(idxf[:], idxu[:, 0:20])
                    psT1 = pmix.tile([20, 128], f32, tag="pmix")
                    nc.tensor.transpose(psT1[:], idxf[:], ident[:])
                    idxT = small.tile([20, 128], f32, tag="idxT")
                    nc.scalar.activation(idxT[:], psT1[:], AF.Copy)
                    psT2 = pmix.tile([16, 8, 20], f32, tag="pmix")
                    for pg in range(8):
                        nc.tensor.transpose(psT2[:, pg, :],
                                            idxT[:, pg * 16:(pg + 1) * 16],
                                            ident[0:20, 0:20])
                    # replicate the 16 index rows to all 128 partitions with
                    # one tiny matmul (rep16 0/1 lhsT) instead of 3 chained
                    # SBUF->SBUF DMAs — shorter critical path to the gathers.
                    idxh = small.tile([16, 160], f16, tag="idxh")
                    nc.scalar.activation(
                        idxh[:].rearrange("q (c pg) -> q pg c", pg=8),
                        psT2[:], AF.Copy)
                    psRep = pmix.tile([128, 160], f32, tag="pmix")
                    nc.tensor.matmul(psRep[:], rep16[:], idxh[:],
                                     start=True, stop=True)
                    idxs16 = small.tile([128, 160], i16, tag="idxs16")
                    nc.scalar.activation(idxs16[:], psRep[:], AF.Copy)
                    dst = dstp.tile([128, K, CO2], gdt, tag="dst", name=f"dst{li}_{t}")
                    for qi, (off, n) in enumerate(GATHER_SPLITS):
                        nc.gpsimd.dma_gather(
                            dst[:, off // 128:(off + n) // 128, :], vtab[:],
                            idxs16[:, off // 16:(off + n) // 16], n, n, CO2,
                            queue_num=qi)
                    ctx2.__exit__(None, None, None)
                    st[t]["dst"] = dst

                def back(t):
                    dst = st.pop(t)["dst"]
                    tsl = slice(t * 128, (t + 1) * 128)
                    if last_layer:
                        # L3: plain-v bf16 rows; dsq via ACT Square.
                        dbf = dst
                        dsq = allp.tile([128, K, CO], bf16, tag="dsq",
                                        name=f"dsq{li}_{t}")
                        nc.scalar.activation(dsq[:], dst[:], AF.Square)
                        vsl = slice(0, CO)
                    elif li == 1:
                        # L2: bf16 [v|v^2] rows used directly.
                        dbf = dst
                        dsq = None
                        vsl = slice(0, CO)
                    else:
                        # L1: f32 [v|v^2] rows -> one bf16 copy of both halves
                        dbf = bfp.tile([128, K, CO2], bf16, tag="dbf")
                        nc.scalar.activation(dbf[:], dst[:], AF.Copy)
                        dsq = None
                        vsl = slice(0, CO)
                    if gdt is bf16:
                        # m = max_j v: 2x bf16 max tree 20->10->5->(2,2,1)->1
                        mt1 = bfp.tile([128, 10, CO], bf16, tag="mt1")
                        nc.vector.tensor_tensor(out=mt1[:], in0=dst[:, 0:10, vsl],
                                                in1=dst[:, 10:20, vsl], op=AOT.max)
                        mt2 = vcp.tile([128, 5, CO], bf16, tag="mt2")
                        nc.vector.tensor_tensor(out=mt2[:], in0=mt1[:, 0:5, :],
                                                in1=mt1[:, 5:10, :], op=AOT.max)
                        nc.vector.tensor_tensor(out=mt2[:, 0:2, :],
                                                in0=mt2[:, 0:2, :],
                                                in1=mt2[:, 2:4, :], op=AOT.max)
                        nc.vector.tensor_tensor(out=mt2[:, 0:1, :],
                                                in0=mt2[:, 0:1, :],
                                                in1=mt2[:, 1:2, :], op=AOT.max)
                        mloc = vcp.tile([128, CO], f32, tag="mloc")
                        nc.vector.tensor_tensor(out=mloc[:],
                                                in0=mt2[:, 0:1, :],
                                                in1=mt2[:, 4:5, :], op=AOT.max)
                    else:
                        mloc = vcp.tile([128, CO], f32, tag="mloc")
                        nc.vector.tensor_reduce(
                            mloc[:], dst[:, :, vsl].rearrange("p j c -> p c j"),
                            mybir.AxisListType.X, AOT.max)
                    # stats matmuls (bf16). For sq_tab layers one jgroup
                    # matmul covers [v|v^2] together (psS row folds to
                    # [Sum v | Sum v^2]); L3 keeps separate psS/psQ passes.
                    first = (t == 0)
                    last = (t == NT - 1)
                    for gi, (j0, gn) in enumerate(jgroups):
                        w = gn * CO2
                        nc.tensor.matmul(psS[:, 0:w], onesCol_bf[:],
                                         dbf[:, j0:j0 + gn, :],
                                         start=(first and gi == 0),
                                         stop=(last and gi == len(jgroups) - 1),
                                         skip_group_check=True)
                    if last_layer:
                        for gi, (j0, gn) in enumerate(jgroups):
                            w = gn * CO2
                            nc.tensor.matmul(psQ[:, 0:w], onesCol_bf[:],
                                             dsq[:, j0:j0 + gn, :],
                                             start=(first and gi == 0),
                                             stop=(last and gi == len(jgroups) - 1),
                                             skip_group_check=True)
                    for h in range(NH):
                        for gi, (j0, gn) in enumerate(jgroups):
                            w = gn * CO2
                            nc.tensor.matmul(
                                psX[h][:, 0:w],
                                ubf_all[:, t, 128 * h:128 * h + CH],
                                dbf[:, j0:j0 + gn, :],
                                start=(first and gi == 0),
                                stop=(last and gi == len(jgroups) - 1),
                                skip_group_check=True)
                    # pre-barrier y: wsum, transpose, stage into yPre / gmax
                    wsum = vcp.tile([128, CO], f32, tag="wsum")
                    nc.vector.tensor_tensor(out=wsum[:], in0=u_all[:, t, :],
                                            in1=mloc[:], op=AOT.add)
                    for h in range(NH):
                        psY = pyp.tile([128, 128], f32, tag="pyp")
                        nc.tensor.transpose(psY[0:CH, :],
                                            wsum[:, 128 * h:128 * h + CH],
                                            ident[:])
                        if not last_layer:
                            nc.scalar.activation(yPre[:, tsl], psY[0:CH, :],
                                                 AF.Copy)
                        else:
                            nc.vector.tensor_tensor(out=gmax[:, h, :],
                                                    in0=gmax[:, h, :],
                                                    in1=psY[0:CH, :], op=AOT.max)

                # tiles 0/1 scores+topk first so the DVE starts immediately;
                # u/v table prep then overlaps their topk on PE/ACT, and
                # plumb(0) lands in the PE queue right after prep (by which
                # time topk(0) has finished -> no head-of-line stall).
                scores_phase(0)
                topk_phase(0)
                scores_phase(1)
                topk_phase(1)

                # u|v computed together per tile: psUV = yB^T[wuv] via the
                # same 3-term bf16 scheme (hi*hi + hi*lo + lo*hi) + bias rows.
                uv_hi, uv_lo = yB[0:CI, :], yB2[0:CI, :]
                for t in range(NT):
                    tsl = slice(t * 128, (t + 1) * 128)
                    psUV = pscore.tile([128, 512], f32, tag="psc",
                                       name=f"psUV{li}_{t}")
                    W = 2 * CO
                    nc.tensor.matmul(psUV[:, 0:W], uv_hi[:, tsl], wuvh[:],
                                     start=True, stop=False)
                    nc.tensor.matmul(psUV[:, 0:W], uv_hi[:, tsl], wuvl[:],
                                     start=False, stop=False)
                    nc.tensor.matmul(psUV[:, 0:W], uv_lo[:, tsl], wuvh[:],
                                     start=False, stop=False)
                    nc.tensor.matmul(psUV[:, 0:W], ones2[:], brw[:],
                                     start=False, stop=True)
                    nc.scalar.activation(u_all[:, t, :], psUV[:, 0:CO], AF.Copy)
                    nc.scalar.activation(ubf_all[:, t, :], psUV[:, 0:CO], AF.Copy)
                    vst = vcp.tile([128, CO2], gdt, tag="vst")
                    nc.scalar.activation(vst[:, 0:CO], psUV[:, CO:W], AF.Copy)
                    if sq_tab:
                        nc.scalar.activation(vst[:, CO:CO2], psUV[:, CO:W],
                                             AF.Square)
                    nc.sync.dma_start(vtab[tsl, :], vst[:])

                # Sum u / Sum u^2 (fp32, exact)
                u_flat = u_all[:].rearrange("p t c -> p (t c)")
                nuv = NT * CO // 512
                for s in range(nuv):
                    usqf = vcp.tile([128, 512], f32, tag="usq")
                    nc.scalar.activation(usqf[:], u_flat[:, 512 * s:512 * (s + 1)],
                                         AF.Square)
                    nc.tensor.matmul(psSu, onesColF[:],
                                     u_flat[:, 512 * s:512 * (s + 1)],
                                     start=(s == 0), stop=(s == nuv - 1),
                                     skip_group_check=True)
                    nc.tensor.matmul(psQu, onesColF[:], usqf[:],
                                     start=(s == 0), stop=(s == nuv - 1),
                                     skip_group_check=True)
                rowSu = rows.tile([1, 512], f32, tag="rowSu")
                nc.scalar.activation(rowSu[:], psSu, AF.Copy)
                rowQu = rows.tile([1, 512], f32, tag="rowQu")
                nc.scalar.activation(rowQu[:], psQu, AF.Copy)
                for row in (rowSu, rowQu):
                    wfull = 512
                    while wfull > CO:
                        half = wfull // 2
                        nc.vector.tensor_tensor(out=row[:, 0:half],
                                                in0=row[:, 0:half],
                                                in1=row[:, half:wfull], op=AOT.add)
                        wfull = half

                psSQ = pstat.tile([33, 512], f32, tag="psSQ", name="psSQm")
                psS = psSQ[0:1, :]
                psQ = psSQ[32:33, :]

                plumb_phase(0)

                # 3-deep pipeline (dst triple-buffered): gathers run two
                # tiles ahead of the stats consumer, so the gather-DMA /
                # back() WAR loop no longer serializes tile pairs.
                # L1 additionally nudges back(t) later on the scheduler's
                # cost-model timeline: the model underestimates the gather
                # DMA, so without the nudge it orders dst-consumers (dbf,
                # mloc, stats) BEFORE the next tiles' idx-plumb ops in each
                # engine queue, and on HW they stall head-of-line on the
                # gather tail, stretching the loop to ~19us/tile.
                for it in range(2, NT + 3):
                    if it - 1 < NT:
                        plumb_phase(it - 1)
                    if it < NT:
                        scores_phase(it)
                    if li == 0 and it >= 3:
                        with tc.tile_wait_until(0.025 + (it - 3) * 0.011):
                            back(it - 3)
                    elif li == 1 and it >= 3:
                        with tc.tile_wait_until(0.260 + (it - 3) * 0.011):
                            back(it - 3)
                    elif it >= 3:
                        back(it - 3)
                    if it < NT:
                        topk_phase(it)

                # ---------- copy out S/Q, then fold 512 -> CO2 ----------
                # sq_tab layers: psS already holds [Sum v | Sum v^2] blocks.
                rowS = rows.tile([1, 512], f32, tag="rowS")
                nc.scalar.activation(rowS[:], psS, AF.Copy)
                if last_layer:
                    rowQ = rows.tile([1, 512], f32, tag="rowQ")
                    nc.scalar.activation(rowQ[:], psQ, AF.Copy)
                    foldrows = (rowS, rowQ)
                else:
                    foldrows = (rowS,)
                for row in foldrows:
                    wfull = 512
                    while wfull > CO2:
                        half = wfull // 2
                        nc.vector.tensor_tensor(out=row[:, 0:half],
                                                in0=row[:, 0:half],
                                                in1=row[:, half:wfull], op=AOT.add)
                        wfull = half
                if not last_layer:
                    rowQ = rowS[:, CO:2 * CO]

                # cross-term: diag of psX via ttr with mask, then -> row
                junk = small.tile([128, 512], f32, tag="junk")
                crossRow = rows.tile([1, 256], f32, tag="crossRow")
                for h in range(NH):
                    ccol = small.tile([128, 1], f32, tag="ccol")
                    nc.vector.tensor_tensor(out=junk[0:CH, :], in0=psX[h][:],
                                            in1=masks[h][:], op=AOT.mult)
                    nc.vector.tensor_reduce(ccol[0:CH, :], junk[0:CH, :],
                                            mybir.AxisListType.X, AOT.add)
                    psCr = pmix.tile([1, CH], f32, tag="pmix")
                    nc.tensor.transpose(psCr[:], ccol[0:CH, :], ident[0:CH, 0:CH])
                    nc.scalar.activation(crossRow[:, 128 * h:128 * h + CH],
                                         psCr[:], AF.Copy)

                # ---------- per-core partial sums -> allreduce ----------
                statsrow = rows.tile([1, 512], f32, tag="statsrow")
                nc.vector.tensor_scalar(out=statsrow[:, 0:CO], in0=rowSu[:, 0:CO],
                                        scalar1=float(K), scalar2=None,
                                        op0=AOT.mult)
                nc.vector.tensor_tensor(out=statsrow[:, 0:CO],
                                        in0=statsrow[:, 0:CO],
                                        in1=rowS[:, 0:CO], op=AOT.add)
                nc.vector.tensor_scalar(out=statsrow[:, CO:2 * CO],
                                        in0=rowQu[:, 0:CO], scalar1=float(K),
                                        scalar2=None, op0=AOT.mult)
                nc.vector.tensor_scalar(out=crossRow[:, 0:CO], in0=crossRow[:, 0:CO],
                                        scalar1=2.0, scalar2=None, op0=AOT.mult)
                nc.vector.tensor_tensor(out=statsrow[:, CO:2 * CO],
                                        in0=statsrow[:, CO:2 * CO],
                                        in1=crossRow[:, 0:CO], op=AOT.add)
                nc.vector.tensor_tensor(out=statsrow[:, CO:2 * CO],
                                        in0=statsrow[:, CO:2 * CO],
                                        in1=rowQ[:, 0:CO], op=AOT.add)

                ccin = dram.tile([1, 2 * CO], f32, tag=f"ccin{li}")
                ccout = dram.tile([1, 2 * CO], f32, tag=f"ccout{li}")
                nc.sync.dma_start(ccin[:], statsrow[:, 0:2 * CO])
                nc.gpsimd.collective_compute(
                    "AllReduce", AOT.add,
                    replica_groups=[list(range(NCORES))],
                    ins=[ccin.opt()], outs=[ccout.opt()])
                statsg = rows.tile([1, 512], f32, tag="statsg")
                nc.sync.dma_start(statsg[:, 0:2 * CO], ccout[:])

                # ---------- BN scale/shift ----------
                cntr = 1.0 / float(B * N * K)
                meanR = rows.tile([1, 256], f32, tag="meanR")
                nc.vector.tensor_scalar(out=meanR[:, 0:CO], in0=statsg[:, 0:CO],
                                        scalar1=cntr, scalar2=None, op0=AOT.mult)
                t1R = rows.tile([1, 256], f32, tag="t1R")
                t2R = rows.tile([1, 256], f32, tag="t2R")
                nc.vector.tensor_scalar(out=t1R[:, 0:CO], in0=statsg[:, CO:2 * CO],
                                        scalar1=cntr, scalar2=None, op0=AOT.mult)
                nc.vector.tensor_tensor(out=t2R[:, 0:CO], in0=meanR[:, 0:CO],
                                        in1=meanR[:, 0:CO], op=AOT.mult)
                nc.vector.tensor_tensor(out=t1R[:, 0:CO], in0=t1R[:, 0:CO],
                                        in1=t2R[:, 0:CO], op=AOT.subtract)
                nc.vector.tensor_scalar(out=t1R[:, 0:CO], in0=t1R[:, 0:CO],
                                        scalar1=1e-5, scalar2=None, op0=AOT.add)
                nc.scalar.activation(t2R[:, 0:CO], t1R[:, 0:CO], AF.Sqrt)
                nc.vector.reciprocal(t1R[:, 0:CO], t2R[:, 0:CO])
                scaleR = rows.tile([1, 256], f32, tag="scaleR")
                nc.vector.tensor_tensor(out=scaleR[:, 0:CO], in0=grow[:].bitcast(f32),
                                        in1=t1R[:, 0:CO], op=AOT.mult)
                shiftR = rows.tile([1, 256], f32, tag="shiftR")
                nc.vector.tensor_tensor(out=shiftR[:, 0:CO], in0=meanR[:, 0:CO],
                                        in1=scaleR[:, 0:CO], op=AOT.mult)
                nc.vector.tensor_tensor(out=shiftR[:, 0:CO], in0=berow[:],
                                        in1=shiftR[:, 0:CO], op=AOT.subtract)

                scol = allp.tile([128, 2], f32, tag="scol")
                tcol = allp.tile([128, 2], f32, tag="tcol")
                for h in range(NH):
                    psc = pmix.tile([128, 1], f32, tag="pmix")
                    nc.tensor.transpose(psc[0:CH, :],
                                        scaleR[:, 128 * h:128 * h + CH],
                                        ident[0:1, 0:1])
                    nc.scalar.activation(scol[0:CH, h:h + 1], psc[0:CH, :], AF.Copy)
                    psc2 = pmix.tile([128, 1], f32, tag="pmix")
                    nc.tensor.transpose(psc2[0:CH, :],
                                        shiftR[:, 128 * h:128 * h + CH],
                                        ident[0:1, 0:1])
                    nc.scalar.activation(tcol[0:CH, h:h + 1], psc2[0:CH, :], AF.Copy)

                # ---------- y-phase (post-barrier): single affine+relu ----------
                if not last_layer:
                    # safe to reuse the single yT buffer: yT's last readers
                    # are this layer's prep (hi/lo split + ysq), long retired
                    # by the time the post-barrier y-phase writes.
                    yTn = ytp.tile([128, N], f32, tag="yt0")
                    nc.scalar.activation(yTn[0:CH, :], yPre[:, :], AF.Relu,
                                         bias=tcol[0:CH, 0:1],
                                         scale=scol[0:CH, 0:1])
                    yT = yTn

            # ---------- head ----------
            psH = pmix.tile([1, 256], f32, tag="pmix")
            for h in range(2):
                gcol = small.tile([128, 1], f32, tag="ccol")
                nc.vector.tensor_reduce(gcol[:], gmax[:, h, :],
                                        mybir.AxisListType.X, AOT.max)
                nc.vector.tensor_scalar(out=gcol[:], in0=gcol[:],
                                        scalar1=scol[:, h:h + 1],
                                        scalar2=tcol[:, h:h + 1],
                                        op0=AOT.mult, op1=AOT.add)
                nc.vector.tensor_scalar_max(gcol[:], gcol[:], 0.0)
                nc.tensor.matmul(psH[:], gcol[:], woT_sb[:, h, :],
                                 start=(h == 0), stop=False,
                                 skip_group_check=True)
            nc.tensor.matmul(psH[:], onesRow[:, 0:1], boRow[:],
                             start=False, stop=True, skip_group_check=True)
            outSb = rows.tile([1, 256], f32, tag="crossRow")
            nc.scalar.activation(outSb[:], psH[:], AF.Copy)
            nc.sync.dma_start(out_ext[:], outSb[:])

    nc.compile()
    return nc


def _host_prep(x, weights):
    """Build per-core input maps. x: [B, N, 3]."""
    import ml_dtypes
    bf = ml_dtypes.bfloat16
    shared = {}
    for li, (ci, co) in enumerate(LAYERS):
        W = np.asarray(weights[f"w{li + 1}"])            # [co, 2*ci]
        wc, wnn = W[:, :ci], W[:, ci:]
        wuv = np.concatenate([(wc - wnn).T, wnn.T], axis=1).astype(np.float32)
        hi = wuv.astype(bf)
        lo = (wuv - hi.astype(np.float32)).astype(bf)
        shared[f"wuvh{li}"] = np.ascontiguousarray(hi)
        shared[f"wuvl{li}"] = np.ascontiguousarray(lo)
        bias = np.asarray(weights[f"b{li + 1}"]).astype(np.float32).reshape(co)
        brw = np.zeros((2, 2 * co), np.float32)
        bhi = bias.astype(bf).astype(np.float32)
        brw[0, :co] = bhi
        brw[1, :co] = bias - bhi
        shared[f"brw{li}"] = brw.astype(bf)
        shared[f"grow{li}"] = np.asarray(weights[f"g{li + 1}"]).reshape(1, co).astype(np.float32)
        shared[f"berow{li}"] = np.asarray(weights[f"be{li + 1}"]).reshape(1, co).astype(np.float32)
        # psX diag mask: v-diagonal within each jgroup block. sq_tab layers
        # (L1/L2) gather [v|v^2] so the block is 2*co wide with the v part
        # first; L3 keeps plain co-wide blocks.
        blk = co if li == len(LAYERS) - 1 else 2 * co
        Gm = 512 // blk
        for h in range(-(-co // 128)):
            hc = min(128, co - 128 * h)
            mk = np.zeros((hc, 512), np.float32)
            for p in range(hc):
                for j in range(Gm):
                    mk[p, j * blk + p + 128 * h] = 1.0
            shared[f"mask{li}_{h}"] = mk
    shared["ident"] = np.eye(128, dtype=np.float32)
    rep = np.zeros((16, 128), np.float16)
    for r in range(16):
        rep[r, r::16] = 1.0
    shared["rep16"] = rep
    shared["woT"] = np.ascontiguousarray(np.asarray(weights["wo"]).T.astype(np.float32))
    shared["boRow"] = np.asarray(weights["bo"]).reshape(1, 256).astype(np.float32)
    ins = []
    for c in range(NCORES):
        m = dict(shared)
        m["xT"] = np.ascontiguousarray(np.asarray(x[c]).T.astype(np.float32))
        ins.append(m)
    return ins


def kernel(**inputs):
    from concourse.bass_utils import run_bass_kernel_spmd
    x = np.asarray(inputs["x"])
    if "nc" not in _BUILT:
        _BUILT["nc"] = _build()
    nc = _BUILT["nc"]
    in_maps = _host_prep(x, inputs)
    res = run_bass_kernel_spmd(nc, in_maps, list(range(NCORES))).results
    out = np.stack([res[c]["out"][0] for c in range(NCORES)], axis=0)
    return out.astype(np.float32)

